# revision 3
# baseline (speedup 1.0000x reference)
"""Trainium2 Bass kernel for Flux single-attention block.

Math (per reference):
  q/k/v = x @ W{q,k,v}.T + b    (x: [S, D], W: [D, D], per-head dim 128)
  q,k: per-head RMSNorm (eps 1e-6, gain g) then interleaved RoPE
  out = softmax(q k^T / sqrt(128)) @ v, non-causal, reshaped [S, H*128]

Sharding: tensor-parallel over heads. 8 cores x 3 heads. Each core gets
replicated x (pre-transposed, bf16), its 1152-row slice of [wq;wk;wv]
(pre-transposed, bf16), biases, and RoPE coefficient tables with the
RMSNorm gains folded in. No collectives.

Per-core kernel:
  Stage B (projection): for each 128-row s-tile, psum[s,1152] accumulated
    over 24 d-tiles (lhsT = xT tile, rhs = WT tile). Epilogue: +bias,
    per-head RMSNorm (ACT Square+accum, Sqrt, DVE reciprocal), RoPE
    (strided rotate-half, cos/sin premultiplied by gains), cast bf16,
    PE-transpose Q,K per head into resident QT/KT [dh, S]; V stays
    natural with a ones-column appended per head (VN [k, h, 129]).
  Stage C (attention): scores computed TRANSPOSED: psum[k-tile, q-chunk]
    = KT_tile^T @ QT (so softmax partials line up for the PV matmul with
    no probs transpose). Exp on ACT (no max subtraction: RMS-normed q,k
    bound |score*scale| <= sqrt(128)). PV: psum[q,129] accumulates
    E^T @ [V | 1]; the ones column yields the softmax denominator in the
    same accumulation. Epilogue: reciprocal + scale, DMA out.
"""

import math
from contextlib import ExitStack

import ml_dtypes
import numpy as np

import concourse.bass as bass
import concourse.tile as tile
from concourse import bacc, mybir
from concourse.bass_utils import run_bass_kernel_spmd
from concourse.masks import make_identity

N_CORES = 8
S = 3072
D = 3072
H = 24
DH = 128
EPS = 1e-6
BF16 = mybir.dt.bfloat16
F32 = mybir.dt.float32
NPBF16 = ml_dtypes.bfloat16


def build_nc(s=S, d=D, hpc=H // N_CORES, n_cores=N_CORES):
    """Build + compile the per-core Bass program (SPMD across n_cores)."""
    P = 128
    ST = s // P          # seq tiles
    DT = d // P          # contraction tiles
    M1 = hpc * DH        # per-projection output cols (q|k|v)
    M = 3 * M1
    QW = min(512, s)     # q-chunk width for scores
    QCH = s // QW        # q-chunks
    QSUB = QW // P       # q-subtiles per chunk
    scale = 1.0 / math.sqrt(DH)

    nc = bacc.Bacc("TRN2", target_bir_lowering=False, debug=False,
                   num_devices=n_cores)

    xt = nc.dram_tensor("xt", [d, s], BF16, kind="ExternalInput").ap()
    wt = nc.dram_tensor("wt", [d, M], BF16, kind="ExternalInput").ap()
    bias = nc.dram_tensor("bias", [M], F32, kind="ExternalInput").ap()
    cq = nc.dram_tensor("cq", [s, DH], F32, kind="ExternalInput").ap()
    sq = nc.dram_tensor("sq", [s, DH], F32, kind="ExternalInput").ap()
    ck = nc.dram_tensor("ck", [s, DH], F32, kind="ExternalInput").ap()
    sk = nc.dram_tensor("sk", [s, DH], F32, kind="ExternalInput").ap()
    out = nc.dram_tensor("out", [s, M1], F32, kind="ExternalOutput").ap()

    xt_r = xt.rearrange("(dt p) s -> p dt s", p=P)      # [128, DT, s]
    wt_r = wt.rearrange("(dt p) m -> p dt m", p=P)      # [128, DT, M]

    with tile.TileContext(nc) as tc, ExitStack() as ctx:
        persist = ctx.enter_context(tc.tile_pool(name="persist", bufs=1))
        QT = persist.tile([P, hpc, s], BF16)     # q^T per head: [dh, s]
        KT = persist.tile([P, hpc, s], BF16)
        VN = persist.tile([P, ST, hpc, DH + 1], BF16)  # [k-part, ktile, h, dh|1]
        bias_bc = persist.tile([P, M], F32)
        ident = persist.tile([P, P], F32)
        make_identity(nc, ident)
        eps_t = persist.tile([P, 1], F32)
        nc.vector.memset(eps_t, float(EPS))
        nc.vector.memset(VN[:, :, :, DH:DH + 1], 1.0)
        nc.gpsimd.dma_start(out=bias_bc, in_=bias[None, :].to_broadcast((P, M)))

        # ---------------- Stage B: projections + norm + rope ----------
        with tc.tile_pool(name="wtp", bufs=1) as wtp, \
             tc.tile_pool(name="xp", bufs=3) as xp, \
             tc.tile_pool(name="cp", bufs=2) as cp, \
             tc.tile_pool(name="bp", bufs=2) as bp, \
             tc.tile_pool(name="psB", bufs=2, space="PSUM") as psB, \
             tc.tile_pool(name="psT", bufs=2, space="PSUM") as psT:

            WT = wtp.tile([P, DT, M], BF16)
            nc.sync.dma_start(WT, wt_r)

            for st in range(ST):
                xts = xp.tile([P, DT, P], BF16, tag="xts")
                nc.sync.dma_start(xts, xt_r[:, :, st * P:(st + 1) * P])

                psq = psB.tile([P, M1], F32, tag="psq")
                psk = psB.tile([P, M1], F32, tag="psk")
                psv = psB.tile([P, M1], F32, tag="psv")
                for dt in range(DT):
                    lhs = xts[:, dt, :]
                    fl = dict(start=(dt == 0), stop=(dt == DT - 1))
                    nc.tensor.matmul(psq, lhs, WT[:, dt, 0:M1], **fl)
                    nc.tensor.matmul(psk, lhs, WT[:, dt, M1:2 * M1], **fl)
                    nc.tensor.matmul(psv, lhs, WT[:, dt, 2 * M1:3 * M1], **fl)

                # V epilogue: bias add + cast, into augmented layout
                for h in range(hpc):
                    nc.vector.tensor_add(
                        VN[:, st, h, 0:DH],
                        psv[:, h * DH:(h + 1) * DH],
                        bias_bc[:, 2 * M1 + h * DH:2 * M1 + (h + 1) * DH])

                # Q/K epilogue
                ssl = slice(st * P, (st + 1) * P)
                for (ps, boff, ct, sn, TT) in (
                        (psq, 0, cq, sq, QT), (psk, M1, ck, sk, KT)):
                    raw = bp.tile([P, M1], F32, tag="raw")
                    nc.vector.tensor_add(raw, ps, bias_bc[:, boff:boff + M1])
                    ssq = bp.tile([P, hpc], F32, tag="ssq")
                    scr = bp.tile([P, M1], F32, tag="scr")
                    for h in range(hpc):
                        nc.scalar.activation(
                            scr[:, h * DH:(h + 1) * DH],
                            raw[:, h * DH:(h + 1) * DH],
                            func=mybir.ActivationFunctionType.Square,
                            accum_out=ssq[:, h:h + 1])
                    rstd = bp.tile([P, hpc], F32, tag="rstd")
                    nc.scalar.activation(rstd, ssq,
                                         func=mybir.ActivationFunctionType.Sqrt,
                                         scale=1.0 / DH, bias=eps_t[:, :])
                    nc.vector.reciprocal(rstd, rstd)
                    qn = bp.tile([P, M1], F32, tag="qn")
                    for h in range(hpc):
                        nc.vector.tensor_scalar_mul(
                            qn[:, h * DH:(h + 1) * DH],
                            raw[:, h * DH:(h + 1) * DH], rstd[:, h:h + 1])
                    # rotate-half: rot[2i] = -qn[2i+1], rot[2i+1] = qn[2i]
                    rot = bp.tile([P, M1], F32, tag="rot")
                    qn3 = qn.rearrange("p (H x two) -> p H x two", H=hpc, two=2)
                    rot3 = rot.rearrange("p (H x two) -> p H x two", H=hpc, two=2)
                    nc.vector.tensor_scalar_mul(rot3[:, :, :, 0], qn3[:, :, :, 1], -1.0)
                    nc.vector.tensor_copy(rot3[:, :, :, 1], qn3[:, :, :, 0])

                    cst = cp.tile([P, DH], F32, tag="c")
                    snt = cp.tile([P, DH], F32, tag="s")
                    nc.sync.dma_start(cst, ct[ssl, :])
                    nc.sync.dma_start(snt, sn[ssl, :])
                    tmp = bp.tile([P, M1], F32, tag="tmp")
                    rts = bp.tile([P, M1], F32, tag="rts")
                    cb = cst[:, None, :].to_broadcast((P, hpc, DH))
                    sb = snt[:, None, :].to_broadcast((P, hpc, DH))
                    t3 = tmp.rearrange("p (H dh) -> p H dh", H=hpc)
                    r3 = rts.rearrange("p (H dh) -> p H dh", H=hpc)
                    nc.vector.tensor_mul(t3, qn.rearrange("p (H dh) -> p H dh", H=hpc), cb)
                    nc.vector.tensor_mul(r3, rot.rearrange("p (H dh) -> p H dh", H=hpc), sb)
                    qf = bp.tile([P, M1], F32, tag="qf")
                    nc.vector.tensor_add(qf, tmp, rts)
                    for h in range(hpc):
                        pst = psT.tile([P, P], F32, tag="pst")
                        nc.tensor.transpose(pst, qf[:, h * DH:(h + 1) * DH], ident)
                        nc.scalar.copy(TT[:, h, ssl], pst)

        # ---------------- Stage C: attention ---------------------------
        with tc.tile_pool(name="ep", bufs=2) as ep, \
             tc.tile_pool(name="op", bufs=3) as op, \
             tc.tile_pool(name="psS", bufs=3, space="PSUM") as psS, \
             tc.tile_pool(name="psO", bufs=4, space="PSUM") as psO:
            for h in range(hpc):
                for qc in range(QCH):
                    qsl = slice(qc * QW, (qc + 1) * QW)
                    E = ep.tile([P, ST, QW], BF16, tag="E")
                    for kt in range(ST):
                        pss = psS.tile([P, QW], F32, tag="pss")
                        nc.tensor.matmul(pss, KT[:, h, kt * P:(kt + 1) * P],
                                         QT[:, h, qsl], start=True, stop=True)
                        nc.scalar.activation(E[:, kt, :], pss,
                                             func=mybir.ActivationFunctionType.Exp,
                                             scale=scale)
                    for qs in range(QSUB):
                        pso = psO.tile([P, DH + 1], F32, tag="pso")
                        for kt in range(ST):
                            nc.tensor.matmul(pso, E[:, kt, qs * P:(qs + 1) * P],
                                             VN[:, kt, h, :],
                                             start=(kt == 0), stop=(kt == ST - 1))
                        rcp = op.tile([P, 1], F32, tag="rcp")
                        nc.vector.reciprocal(rcp, pso[:, DH:DH + 1])
                        osb = op.tile([P, DH], F32, tag="osb")
                        nc.vector.tensor_scalar_mul(osb, pso[:, 0:DH], rcp)
                        r0 = qc * QW + qs * P
                        nc.sync.dma_start(out[r0:r0 + P, h * DH:(h + 1) * DH], osb)

    nc.compile()
    return nc


def prep_in_maps(hidden_states, freqs_cos, freqs_sin, wq, bq, wk, bk, wv, bv,
                 gq, gk, n_cores=N_CORES, hpc=H // N_CORES):
    """Host-side sharding/layout prep. Returns per-core input maps."""
    x = np.asarray(hidden_states, np.float32).reshape(-1, np.asarray(hidden_states).shape[-1])
    cos = np.asarray(freqs_cos, np.float32)
    sin = np.asarray(freqs_sin, np.float32)
    gq = np.asarray(gq, np.float32)
    gk = np.asarray(gk, np.float32)
    dh = cos.shape[1]

    xt_bf = np.ascontiguousarray(x.T).astype(NPBF16)

    def swap_pairs(g):
        return np.ascontiguousarray(g.reshape(-1, 2)[:, ::-1]).reshape(-1)

    cqh = np.ascontiguousarray(cos * gq[None, :])
    sqh = np.ascontiguousarray(sin * swap_pairs(gq)[None, :])
    ckh = np.ascontiguousarray(cos * gk[None, :])
    skh = np.ascontiguousarray(sin * swap_pairs(gk)[None, :])

    m1 = hpc * dh
    in_maps = []
    for c in range(n_cores):
        rs = slice(c * m1, (c + 1) * m1)
        wcat = np.concatenate([wq[rs], wk[rs], wv[rs]], axis=0)
        wt_bf = np.ascontiguousarray(np.asarray(wcat, np.float32).T).astype(NPBF16)
        bcat = np.concatenate([bq[rs], bk[rs], bv[rs]]).astype(np.float32)
        in_maps.append({
            "xt": xt_bf, "wt": wt_bf, "bias": bcat,
            "cq": cqh, "sq": sqh, "ck": ckh, "sk": skh,
        })
    return in_maps


_NC_CACHE = {}


def _get_nc():
    if "nc" not in _NC_CACHE:
        _NC_CACHE["nc"] = build_nc()
    return _NC_CACHE["nc"]


def kernel(**inputs) -> np.ndarray:
    nc = _get_nc()
    in_maps = prep_in_maps(**inputs)
    res = run_bass_kernel_spmd(nc, in_maps, core_ids=list(range(N_CORES)))
    full = np.concatenate([res.results[c]["out"] for c in range(N_CORES)], axis=1)
    return full.reshape(1, S, H * DH).astype(np.float32)
